# revision 27
# baseline (speedup 1.0000x reference)
"""Channel-attention kernel for Trainium2 (8 NeuronCores, data-parallel over batch).

Math: the reference expands x (B,C,T,1) to 8 channels via a 1x1 conv before the
Q@K^T einsum.  Algebraically, with alpha=w1.w2, beta=w1.b2, delta=b1.w2,
eta=b1.b2 and s[b,c]=sum_t x[b,c,t]:

    energy[b,c,e] = alpha*G[b,c,e] + beta*s[b,c] + delta*s[b,e] + T*eta
    G[b] = X[b] @ X[b]^T          (X[b] = x[b,:,:,0], shape (C,T))

The beta*s[c] and T*eta terms are constant along the e (last) axis, so they
cancel in the min-max normalization; only alpha*G + delta*s_e matters.  This
cuts the contraction from T*8 down to T (the advertised 8x headroom).

Per core: 8 batches, processed as 4 pairs of 2 batches stacked on the 128
partitions.  Per pair:
  - load X2 (128,4000) f32, cast to bf16 (zero-padded to 4096 cols)
  - xbar DMA-transpose -> Xt: 32 tiles of (t=128, c=128), each with a ones
    column appended (so the Gram matmuls also produce the row-sums s)
  - Gram matmuls (bf16): psum (128,129) accumulates [G2 | s]
  - s-row via PE transpose; aux matmul adds (delta/alpha)*s_row to psum
  - extract diagonal (64,64) blocks scaled by alpha, min-max norm + softmax
  - attention (block-diag, scaled by gamma) @ X2bf -> psum, + x, store
"""

import numpy as np
import ml_dtypes
from contextlib import ExitStack

import concourse.bass as bass
import concourse.tile as tile
from concourse import mybir
from concourse.bass_utils import run_bass_kernel_spmd
from concourse.alu_op_type import AluOpType

F32 = mybir.dt.float32
BF16 = mybir.dt.bfloat16
AX = mybir.AxisListType.X

B, C, T = 64, 64, 4000
NCORES = 8
BPC = B // NCORES          # 8 batches per core
PAIRS = BPC // 2           # 4 pairs of 2 batches
ROWS = BPC * C             # 512 rows of (C,T) per core
TP = 4096                  # T padded to a multiple of 128
NKT = TP // 128            # 32 k-tiles
NCHUNK = 8
CHW = T // NCHUNK          # 500 (fits one PSUM bank in f32)
EPS = 1e-8
TSTRIDE = 129              # per-k-tile stride in Xt (128 data cols + 1 ones col)
NTSPLIT = 2                # dma-transposes per pair (pipelining granularity)


def _body(ctx, tc, out_ap, x_ap, idf_ap, alpha, doa, gamma):
    nc = tc.nc

    singles = ctx.enter_context(tc.tile_pool(name="singles", bufs=1))
    xfp = ctx.enter_context(tc.tile_pool(name="xfp", bufs=3))
    xbp = ctx.enter_context(tc.tile_pool(name="xbp", bufs=3))
    xtp = ctx.enter_context(tc.tile_pool(name="xtp", bufs=3))
    obp = ctx.enter_context(tc.tile_pool(name="obp", bufs=2))
    attp = ctx.enter_context(tc.tile_pool(name="attp", bufs=2))
    stage = ctx.enter_context(tc.tile_pool(name="stage", bufs=2))
    smalls = ctx.enter_context(tc.tile_pool(name="smalls", bufs=3))

    ps_s = ctx.enter_context(tc.tile_pool(name="ps_s", bufs=2, space="PSUM"))
    ps_g = ctx.enter_context(tc.tile_pool(name="ps_g", bufs=2, space="PSUM"))
    ps_o = ctx.enter_context(tc.tile_pool(name="ps_o", bufs=4, space="PSUM"))

    ident_f32 = singles.tile([128, 128], F32)
    nc.sync.dma_start(ident_f32[:], idf_ap)
    ones_row = singles.tile([1, 128], BF16)
    nc.vector.memset(ones_row[:], 1.0)
    # preload the ACT function tables during the ramp
    warm_act = singles.tile([1, 2], F32)
    nc.scalar.activation(
        warm_act[:], ones_row[0:1, 0:2], mybir.ActivationFunctionType.Exp
    )

    st = [{} for _ in range(PAIRS)]

    def stage0(p):
        """loads (kept at the head of the HWDGE ring, no head-of-line)."""
        v = st[p]
        rows = slice(p * 128, (p + 1) * 128)
        x_f32 = xfp.tile([128, T], F32)
        half = T // 2
        nc.sync.dma_start(x_f32[:, 0:half], x_ap[rows, 0:half])
        nc.sync.dma_start(x_f32[:, half:T], x_ap[rows, half:T])
        v["x_f32"] = x_f32

    def stage1(p):
        """cast (ACT, with row-sum accumulation) + DMA transposes + the
        s-row prep (so the PE stream later never stalls on it)."""
        v = st[p]
        x_f32 = v["x_f32"]
        x_bf = xbp.tile([128, TP], BF16)
        half = TP // 2  # aligned with the transpose halves
        xt = xtp.tile([128, NKT * 128], BF16)
        ksp = NKT // NTSPLIT

        def transp(h):
            nc.sync.dma_start_transpose(
                xt[:, h * ksp * 128:(h + 1) * ksp * 128].rearrange(
                    "q (k f) -> q k f", f=128
                ),
                x_bf[:, h * ksp * 128:(h + 1) * ksp * 128],
            )

        # the two cast halves run concurrently on DVE and ACT, each also
        # accumulating its half of the row sums s
        s_ab = smalls.tile([128, 2], F32)
        nc.vector.tensor_scalar(
            x_bf[:, 0:half], x_f32[:, 0:half], scalar1=1.0, scalar2=0.0,
            op0=AluOpType.mult, op1=AluOpType.add, accum_out=s_ab[:, 0:1],
        )
        transp(0)
        nc.scalar.activation(
            x_bf[:, half:T], x_f32[:, half:T],
            mybir.ActivationFunctionType.Copy, accum_out=s_ab[:, 1:2],
        )
        nc.vector.memset(x_bf[:, T:TP], 0.0)
        transp(1)
        s_col = smalls.tile([128, 1], F32)
        nc.vector.tensor_reduce(s_col[:], s_ab[:], axis=AX, op=AluOpType.add)
        st_ps = ps_s.tile([1, 128], F32, tag="st")
        nc.tensor.transpose(st_ps[:], s_col[:], ident_f32[:])
        rhs_aux = smalls.tile([1, 128], BF16)
        nc.vector.tensor_scalar_mul(rhs_aux[:], st_ps[:], doa)
        v.update(x_bf=x_bf, xt=xt, rhs_aux=rhs_aux)

    def stage2x(p):
        """Gram matmuls + the aux rank-1 update (pure PE, no stalls)."""
        v = st[p]
        xt = v["xt"]
        psum_g = ps_g.tile([128, 128], F32, tag="g")
        for kt in range(NKT):
            base = kt * 128
            nc.tensor.matmul(
                psum_g[:],
                lhsT=xt[:, base: base + 128],
                rhs=xt[:, base: base + 128],
                start=(kt == 0),
                stop=(kt == NKT - 1),
            )
        nc.tensor.matmul(
            psum_g[:],
            lhsT=ones_row[:],
            rhs=v["rhs_aux"][:],
            start=False,
            stop=True,
            skip_group_check=True,
        )
        v["psum_g"] = psum_g

    def stage2y(p):
        """energy extraction + min-max softmax -> attention lhsT."""
        v = st[p]
        psum_g = v["psum_g"]
        # Diagonal (64,64) blocks, scaled by alpha -> energy (128, 64)
        e_sb = smalls.tile([128, 64], F32)
        nc.vector.tensor_scalar_mul(e_sb[0:64, :], psum_g[0:64, 0:64], alpha)
        nc.vector.tensor_scalar_mul(
            e_sb[64:128, :], psum_g[64:128, 64:128], alpha
        )

        # min-max normalize along free axis, then softmax (normalized values
        # live in [0,1], so no max-subtraction is needed before exp)
        rmax = smalls.tile([128, 1], F32)
        nc.vector.tensor_reduce(rmax[:], e_sb[:], axis=AX, op=AluOpType.max)
        rmin = smalls.tile([128, 1], F32)
        nc.vector.tensor_reduce(rmin[:], e_sb[:], axis=AX, op=AluOpType.min)
        den = smalls.tile([128, 1], F32)
        nc.vector.tensor_scalar(
            den[:], rmax[:], scalar1=rmin[:], scalar2=EPS,
            op0=AluOpType.subtract, op1=AluOpType.add,
        )
        rden = smalls.tile([128, 1], F32)
        nc.vector.reciprocal(rden[:], den[:])
        nbias = smalls.tile([128, 1], F32)
        nc.vector.scalar_tensor_tensor(
            nbias[:], in0=rmin[:], scalar=-1.0, in1=rden[:],
            op0=AluOpType.mult, op1=AluOpType.mult,
        )
        ex = smalls.tile([128, 64], F32)
        nc.scalar.activation(
            ex[:], e_sb[:], mybir.ActivationFunctionType.Exp,
            bias=nbias[:], scale=rden[:],
        )
        ssum = smalls.tile([128, 1], F32)
        nc.vector.tensor_reduce(ssum[:], ex[:], axis=AX, op=AluOpType.add)
        rsum = smalls.tile([128, 1], F32)
        nc.vector.reciprocal(rsum[:], ssum[:])

        latt = attp.tile([128, 128], BF16)
        nc.vector.memset(latt[:], 0.0)
        nc.vector.tensor_scalar(
            latt[0:64, 0:64], ex[0:64, :], scalar1=rsum[0:64], scalar2=gamma,
            op0=AluOpType.mult, op1=AluOpType.mult,
        )
        nc.vector.tensor_scalar(
            latt[64:128, 64:128], ex[64:128, :], scalar1=rsum[64:128],
            scalar2=gamma, op0=AluOpType.mult, op1=AluOpType.mult,
        )
        v["latt"] = latt

    def stage3(p):
        """attended chunks + residual add + store.  Even chunks: DVE adds
        from PSUM (store each immediately).  Odd chunks: ACT copies
        PSUM->SBUF, one GPSIMD op adds them all, one strided store."""
        v = st[p]
        rows = slice(p * 128, (p + 1) * 128)
        x_f32, x_bf, latt = v["x_f32"], v["x_bf"], v["latt"]
        out_sb = obp.tile([128, T], F32)
        att_st = stage.tile([128, 4, CHW], F32)
        last = p == PAIRS - 1
        for ch in range(NCHUNK):
            cols = slice(ch * CHW, (ch + 1) * CHW)
            psum_o = ps_o.tile([128, CHW], F32, tag="o")
            nc.tensor.matmul(
                psum_o[:], lhsT=latt[:], rhs=x_bf[:, cols], start=True,
                stop=True,
            )
            if ch % 2 == 0 or last:
                nc.vector.tensor_add(out_sb[:, cols], psum_o[:], x_f32[:, cols])
            else:
                nc.scalar.copy(att_st[:, ch // 2, :], psum_o[:])
        ev3 = out_sb.rearrange("q (c w) -> q c w", w=CHW)[:, 0::2, :]
        od3 = out_ap[rows, :].rearrange("q (c w) -> q c w", w=CHW)
        if last:
            nc.sync.dma_start(out_ap[rows, :], out_sb[:])
        else:
            nc.sync.dma_start(od3[:, 0::2, :], ev3)
            oddv = out_sb.rearrange("q (c w) -> q c w", w=CHW)[:, 1::2, :]
            xodd = x_f32.rearrange("q (c w) -> q c w", w=CHW)[:, 1::2, :]
            nc.gpsimd.tensor_add(oddv, att_st[:], xodd)
            nc.sync.dma_start(od3[:, 1::2, :], oddv)
        v.clear()

    # software-pipelined schedule, hand-skewed so the PE instruction stream
    # (st-transpose / gram+aux / att) never waits on same-pair DVE/ACT work
    sched = [
        (stage0, 0), (stage1, 0), (stage0, 1), (stage1, 1),
        (stage2x, 0), (stage0, 2), (stage1, 2),
        (stage2x, 1), (stage2y, 0), (stage0, 3), (stage1, 3),
        (stage2x, 2), (stage3, 0), (stage2y, 1),
        (stage2x, 3), (stage3, 1), (stage2y, 2),
        (stage3, 2), (stage2y, 3),
        (stage3, 3),
    ]
    for fn, p in sched:
        fn(p)


_MODULE_CACHE = {}


def _build_module(alpha, doa, gamma):
    key = (alpha, doa, gamma)
    if key in _MODULE_CACHE:
        return _MODULE_CACHE[key]
    nc = bass.Bass(
        "TRN2", target_bir_lowering=False, debug=False, num_devices=NCORES
    )
    x_ap = nc.dram_tensor("x", (ROWS, T), F32, kind="ExternalInput").ap()
    idf_ap = nc.dram_tensor("idf", (128, 128), F32, kind="ExternalInput").ap()
    out_ap = nc.dram_tensor("out", (ROWS, T), F32, kind="ExternalOutput").ap()
    with tile.TileContext(nc) as tc, ExitStack() as ctx:
        _body(ctx, tc, out_ap, x_ap, idf_ap, alpha, doa, gamma)
    if _LEGALIZE_WAITS:
        _split_waits(nc)
    _MODULE_CACHE[key] = nc
    return nc


# The wait-split legalization confuses CoreSim's bookkeeping (hand-built
# NoOps bypass nc.inst_map); tests flip this off for simulation runs.
_LEGALIZE_WAITS = True


def _split_waits(nc):
    """walrus TRN2 codegen allows only ONE sync wait per instruction; when
    Tile emits more (e.g. PSUM slot reuse: previous-writer completion +
    previous-reader), hoist the extras onto same-engine NoOps inserted
    immediately before — the sequencer dispatches in order, so the blocking
    semantics are identical."""
    nid = [0]
    for f in nc.m.functions:
        for block in f.blocks:
            out = []
            for inst in block.instructions:
                si = getattr(inst, "sync_info", None)
                if (
                    si is not None
                    and si.on_wait
                    and len(si.on_wait) > 1
                    and type(inst).__name__ != "InstNoOp"
                ):
                    waits = list(si.on_wait)
                    for w in waits[:-1]:
                        nid[0] += 1
                        out.append(
                            mybir.InstNoOp(
                                name=f"{inst.name}-wsplit{nid[0]}",
                                engine=inst.engine,
                                ins=[],
                                outs=[],
                                sync_info=mybir.SyncInfo(
                                    on_wait=[w], on_update=[]
                                ),
                                text_hint="wait-split",
                                bass_nofuse=True,
                            )
                        )
                    inst.sync_info = mybir.SyncInfo(
                        on_wait=waits[-1:], on_update=list(si.on_update)
                    )
                out.append(inst)
            block.instructions[:] = out


def _prepare(inputs):
    x = np.ascontiguousarray(
        np.asarray(inputs["x"], dtype=np.float32).reshape(B * C, T)
    )
    w1 = np.asarray(inputs["w1"], dtype=np.float64)
    b1 = np.asarray(inputs["b1"], dtype=np.float64)
    w2 = np.asarray(inputs["w2"], dtype=np.float64)
    b2 = np.asarray(inputs["b2"], dtype=np.float64)
    gamma = float(np.asarray(inputs["gamma"]))
    alpha = float(w1 @ w2)
    delta = float(b1 @ w2)
    assert abs(alpha) > 1e-12, "degenerate alpha not supported"
    nc = _build_module(alpha, delta / alpha, gamma)
    ident_f = np.eye(128, dtype=np.float32)
    in_maps = [
        {"x": x[i * ROWS:(i + 1) * ROWS], "idf": ident_f}
        for i in range(NCORES)
    ]
    return nc, in_maps


def kernel(**inputs):
    nc, in_maps = _prepare(inputs)
    res = run_bass_kernel_spmd(nc, in_maps, core_ids=list(range(NCORES)))
    out = np.concatenate([res.results[i]["out"] for i in range(NCORES)], axis=0)
    return out.reshape(B, C, T, 1)
